# revision 14
# baseline (speedup 1.0000x reference)
"""Trainium2 Bass kernel for nn_AdaptiveMultiGabor2DLayer (v3).

Math (per channel c, ic = indices[c]):
    lin_l = x[c] @ W_l[ic] + b_l[ic]   (complex [NPTS, OUT_F]), l = 1,2
    out[c] = exp(i*30*lin1 - 25|lin1|^2 - 25|lin2|^2)
with p = 5Re lin1, q~ = 5Im lin1 + 3, r = 5Re lin2, s = 5Im lin2:
    out = C * erf'(p)erf'(q~)erf'(r)erf'(s) * (cos 6p + i sin 6p),
    C = e^9 (sqrt(pi)/2)^4,  erf'(z) = (2/sqrt(pi)) e^{-z^2}.

Structure per core (8 channels, expert parallel, no collectives):
  * lin1 (phase-critical) in bf16: P,Q directly in PSUM via K=512
    re/im-interleaved matmuls (4 chunks), exactly as the proven baseline.
  * lin2 (enters |.|^2 only - fp8 error contributes O(2 r dr) ~ 1e-3) via
    fp8e4 DoubleRow matmuls at half the PE cycles; weights pre-scaled by
    64*5 (fp8 subnormal avoidance), undone by the erf' scale=1/64.
  * Epilogue: 4x Derivative_Erf straight from PSUM (fused square+exp+bias),
    3 DVE 16-bit products for the gaussian envelope (e^9 folded via one
    scalar_tensor_tensor), full-angle trig: theta = 6P + 30Re(b1) staged
    in fp16, range-wrapped into [-pi,pi] by a 2x-enabled ADD_RANGE_WRAP
    clone (theta reaches +-4.4 rad, beyond the Sin table), one Sin each
    for sin/cos, and the two E*sin/E*cos products on the otherwise-idle
    GPSIMD engine.
  * ACT table sets: erf_derivative (G phase) / trig_and_small (T phase),
    batched over CH_BATCH channels -> only 2 table loads per batch.
"""

import math
import sys

import numpy as np

NCORES = 8
NCHAN = 64
NPTS = 2048
IN_F = 256
OUT_F = 256
CH_PER_CORE = NCHAN // NCORES
CH_BATCH = 3
PSW = 1024                 # psum quantity tile width (2 banks)
FP8_SCALE = 64.0
CBIG = math.exp(9.0) * (math.sqrt(math.pi) / 2.0) ** 4


def _ensure_path():
    try:
        import concourse  # noqa: F401
    except ImportError:
        for p in ("/opt/trn_rl_repo", "/root/.axon_site/_ro/trn_rl_repo"):
            if p not in sys.path:
                sys.path.insert(0, p)


_OPS_CACHE = {}


def _register_op(name, spec, perf_en=True):
    from concourse import dve_ops
    from concourse.dve_spec import _has_src1, lower
    from concourse.dve_uop import DveOpSpec

    existing = [o for o in dve_ops.OPS if o.name == name]
    if existing:
        return existing[0]
    row = max(dve_ops._SUB_OPCODE_FOR_NAME.values()) + 1
    assert row < 0x20
    dve_ops._SUB_OPCODE_FOR_NAME[name] = row
    shas = {}
    for ver in ("v3",):
        uops = lower(spec, ver=ver)
        shas[ver] = DveOpSpec(
            name=name, opcode=row, uops=uops, rd1_en=_has_src1(spec)
        ).sha(ver)
    op = dve_ops.DveOp(name, spec, subdim=False, uops_sha=shas,
                       perf_en={"v3": perf_en})
    dve_ops.OPS.append(op)
    dve_ops.CUSTOM_DVE_SPECS[name] = spec
    return op


def _get_arw():
    """ADD_RANGE_WRAP clone with the 16-bit perf slot enabled."""
    if "arw" in _OPS_CACHE:
        return _OPS_CACHE["arw"]
    _ensure_path()
    from concourse import dve_ops

    arw_op = _register_op("ARW2X_ANT", dve_ops.ADD_RANGE_WRAP.spec)
    _OPS_CACHE["arw"] = arw_op
    return arw_op


_NC_CACHE = {}


def build_nc(nch=CH_PER_CORE, npts=NPTS, ch_batch=CH_BATCH):
    key = (nch, npts, ch_batch)
    if key in _NC_CACHE:
        return _NC_CACHE[key]
    _ensure_path()
    import concourse.bacc as bacc
    import concourse.tile as tile
    from concourse import mybir

    arw_op = _get_arw()

    dt = mybir.dt
    AF = mybir.ActivationFunctionType
    OP = mybir.AluOpType
    PM = mybir.MatmulPerfMode

    n_pw = npts // PSW
    inv8 = 1.0 / FP8_SCALE

    nc = bacc.Bacc("TRN2", target_bir_lowering=False)
    xt_d = nc.declare_dram_parameter("xt", [nch, 512, npts], dt.bfloat16, isOutput=False)
    xq_d = nc.declare_dram_parameter("xq8", [nch, 2, 128, 2, npts], dt.float8e4, isOutput=False)
    w1_d = nc.declare_dram_parameter("w1", [nch, 4, 128, 512], dt.bfloat16, isOutput=False)
    w2_d = nc.declare_dram_parameter("w2", [nch, 2, 128, 2, 512], dt.float8e4, isOutput=False)
    bv_d = nc.declare_dram_parameter("biasv", [nch, 6, 256], dt.float32, isOutput=False)
    out_d = nc.declare_dram_parameter("out", [nch, 2, 256, npts], dt.bfloat16, isOutput=True)

    def bcol(ch, v, jh):
        return (ch * 6 + v) * 2 + jh

    n_stage = ch_batch * 2

    with tile.TileContext(nc) as tc:
        with (
            tc.tile_pool(name="xpool", bufs=2) as xpool,
            tc.tile_pool(name="wpool", bufs=2) as wpool,
            tc.tile_pool(name="cpool", bufs=1) as cpool,
            tc.tile_pool(name="spool", bufs=2) as spool,
            tc.tile_pool(name="stpool", bufs=1) as stpool,
            tc.tile_pool(name="pspool", bufs=2, space="PSUM") as pspool,
            tc.tile_pool(name="ps2pool", bufs=4, space="PSUM") as ps2pool,
        ):
            biast = cpool.tile([128, nch * 6 * 2], dt.float32)
            nc.sync.dma_start(
                out=biast[:], in_=bv_d[:].rearrange("c v (h p) -> p (c v h)", p=128)
            )

            def chain(inst):
                tc.chain_iter_dep("act_order", inst.ins if hasattr(inst, "ins") else inst)

            _cdma = {}

            def load_channel(ch):
                if ch in _cdma:
                    return _cdma[ch]
                xt_ks = [
                    xpool.tile([128, npts], dt.bfloat16, tag=f"xt{kc}", name=f"xt{kc}_{ch}")
                    for kc in range(4)
                ]
                xq = [xpool.tile([128, 2, npts], dt.float8e4, tag=f"xq{c}", name=f"xq{c}_{ch}")
                      for c in range(2)]
                w1 = wpool.tile([128, 4, 512], dt.bfloat16, tag="w1", name=f"w1_{ch}")
                w2 = wpool.tile([128, 2, 2, 512], dt.float8e4, tag="w2", name=f"w2_{ch}")
                nc.sync.dma_start(out=xt_ks[0][:], in_=xt_d[ch, 0:128, :])
                nc.sync.dma_start(
                    out=w1[:], in_=w1_d[ch].rearrange("c p j -> p c j")
                )
                for kc in range(1, 4):
                    nc.sync.dma_start(
                        out=xt_ks[kc][:], in_=xt_d[ch, kc * 128: kc * 128 + 128, :]
                    )
                for c in range(2):
                    nc.sync.dma_start(out=xq[c][:], in_=xq_d[ch, c])
                nc.sync.dma_start(
                    out=w2[:], in_=w2_d[ch].rearrange("c p i j -> p c i j")
                )
                _cdma[ch] = (xt_ks, xq, w1, w2)
                return _cdma[ch]

            def emit_unit_G(ch, jh, staged):
                xt_ks, xq, w1, w2 = load_channel(ch)
                th = stpool.tile([128, npts], dt.float16, tag="th", bufs=n_stage,
                                 name=f"th_{ch}_{jh}")
                mh = stpool.tile([128, npts], dt.bfloat16, tag="mh", bufs=n_stage,
                                 name=f"mh_{ch}_{jh}")
                g = [spool.tile([128, npts], dt.bfloat16, tag=f"g{qi}",
                                name=f"g{qi}_{ch}_{jh}") for qi in range(4)]
                for pw in range(n_pw):
                    psl = slice(pw * PSW, pw * PSW + PSW)
                    # P, Q: bf16 K=512 (4 chunks); j cols jh*128 (P) and
                    # 256 + jh*128 (Q) of the [512]-wide lin1 weight matrix
                    for qi, cb in ((0, jh * 128), (1, 256 + jh * 128)):
                        ps = pspool.tile([128, PSW], dt.float32, tag="q",
                                         name=f"ps{qi}_{ch}_{jh}_{pw}")
                        for ns in range(PSW // 512):
                            msl = slice(pw * PSW + ns * 512, pw * PSW + ns * 512 + 512)
                            for kc in range(4):
                                nc.tensor.matmul(
                                    ps[:, ns * 512: ns * 512 + 512],
                                    w1[:, kc, cb: cb + 128], xt_ks[kc][:, msl],
                                    start=(kc == 0), stop=(kc == 3))
                        inst = nc.scalar.activation(
                            g[qi][:, psl], ps[:], AF.Derivative_Erf,
                            bias=biast[:, bcol(ch, qi, jh): bcol(ch, qi, jh) + 1])
                        chain(inst)
                        if qi == 0:
                            # theta = 6*P + 30Re(b1)  [fp16]
                            nc.vector.tensor_scalar(
                                out=th[:, psl], in0=ps[:], scalar1=6.0,
                                scalar2=biast[:, bcol(ch, 4, jh): bcol(ch, 4, jh) + 1],
                                op0=OP.mult, op1=OP.add)
                    # R, S: fp8 DoubleRow K=512 (2 chunks of 2x128),
                    # bank-aligned [128,512] psum tiles (DR at a psum column
                    # offset faults the exec unit)
                    for qi, cb in ((2, jh * 128), (3, 256 + jh * 128)):
                        for ns in range(PSW // 512):
                            msl = slice(pw * PSW + ns * 512, pw * PSW + ns * 512 + 512)
                            ps = ps2pool.tile([128, 512], dt.float32, tag="q2",
                                              name=f"ps{qi}_{ch}_{jh}_{pw}_{ns}")
                            for c in range(2):
                                nc.tensor.matmul(
                                    ps[:], w2[:, c, :, cb: cb + 128], xq[c][:, :, msl],
                                    start=(c == 0), stop=(c == 1),
                                    perf_mode=PM.DoubleRow)
                            inst = nc.scalar.activation(
                                g[qi][:, pw * PSW + ns * 512: pw * PSW + ns * 512 + 512], ps[:],
                                AF.Derivative_Erf, scale=inv8,
                                bias=biast[:, bcol(ch, qi, jh): bcol(ch, qi, jh) + 1])
                            chain(inst)
                # envelope: mh = C * g0 g1 g2 g3  [16-bit 2x DVE, full width]
                m1 = spool.tile([128, npts], dt.bfloat16, tag="m1", name=f"m1_{ch}_{jh}")
                m2 = spool.tile([128, npts], dt.bfloat16, tag="m2", name=f"m2_{ch}_{jh}")
                nc.vector.tensor_tensor(out=m1[:], in0=g[0][:], in1=g[1][:], op=OP.mult)
                nc.vector.tensor_tensor(out=m2[:], in0=g[2][:], in1=g[3][:], op=OP.mult)
                nc.vector.scalar_tensor_tensor(
                    out=mh[:], in0=m1[:], scalar=CBIG, in1=m2[:],
                    op0=OP.mult, op1=OP.mult)
                staged.append((ch, jh, th, mh))

            def emit_unit_T(ch, jh, th, mh):
                # sin directly on theta (|theta| <= 4.44; the Sin table is
                # good past 3.8 for all but ~1e-5 of elements, negligible in
                # the norm); cos needs theta+pi/2 (up to 6.0) -> one wrap.
                thc = spool.tile([128, npts], dt.float16, tag="thc", name=f"thc_{ch}_{jh}")
                nc.vector._custom_dve(arw_op, out=thc[:], in0=th[:], s0=math.pi / 2,
                                      s1=math.pi, imm2=2.0 * math.pi)
                sn = spool.tile([128, npts], dt.bfloat16, tag="sn", name=f"sn_{ch}_{jh}")
                cs = spool.tile([128, npts], dt.bfloat16, tag="cs", name=f"cs_{ch}_{jh}")
                zb = biast[:, bcol(ch, 5, jh): bcol(ch, 5, jh) + 1]
                inst = nc.scalar.activation(sn[:], th[:], AF.Sin, bias=zb)
                chain(inst)
                inst = nc.scalar.activation(cs[:], thc[:], AF.Sin, bias=zb)
                chain(inst)
                oii = spool.tile([128, npts], dt.bfloat16, tag="oii", name=f"oii_{ch}_{jh}")
                orr = spool.tile([128, npts], dt.bfloat16, tag="orr", name=f"orr_{ch}_{jh}")
                nc.gpsimd.tensor_tensor(out=oii[:], in0=mh[:], in1=sn[:], op=OP.mult)
                nc.gpsimd.tensor_tensor(out=orr[:], in0=mh[:], in1=cs[:], op=OP.mult)
                nc.sync.dma_start(out=out_d[ch, 0, jh * 128: jh * 128 + 128, :], in_=orr[:])
                nc.sync.dma_start(out=out_d[ch, 1, jh * 128: jh * 128 + 128, :], in_=oii[:])

            for b0 in range(0, nch, ch_batch):
                chs = list(range(b0, min(b0 + ch_batch, nch)))
                staged = []
                for ch in chs:
                    for jh in range(2):
                        emit_unit_G(ch, jh, staged)
                for (ch, jh, th, mh) in staged:
                    emit_unit_T(ch, jh, th, mh)

    nc.finalize()
    _NC_CACHE[key] = nc
    return nc


def prepare_inputs(x, indices, U1, V1, b1, U2, V2, b2):
    """Host marshaling. lin1: baseline interleaved-K bf16 form. lin2: fp8
    DoubleRow form scaled by 64*5."""
    import ml_dtypes

    bf16 = ml_dtypes.bfloat16
    f8 = ml_dtypes.float8_e4m3
    x = np.asarray(x)
    indices = np.asarray(indices).astype(np.int64)
    U1 = np.asarray(U1); V1 = np.asarray(V1); b1 = np.asarray(b1)
    U2 = np.asarray(U2); V2 = np.asarray(V2); b2 = np.asarray(b2)
    nch, npts, inf = x.shape

    # lin1 weights: [2*inf interleaved, 2*OUT_F] bf16, x5
    W1 = (U1[indices] @ V1).reshape(nch, inf, OUT_F)
    B = np.empty((nch, 2 * inf, 2 * OUT_F), np.float32)
    Wr = 5.0 * np.ascontiguousarray(W1.real)
    Wi = 5.0 * np.ascontiguousarray(W1.imag)
    B[:, 0::2, :OUT_F] = Wr
    B[:, 1::2, :OUT_F] = -Wi
    B[:, 0::2, OUT_F:] = Wi
    B[:, 1::2, OUT_F:] = Wr
    w1 = np.ascontiguousarray(B.reshape(nch, 4, 128, 2 * OUT_F)).astype(bf16)

    # lin2 weights: blocked [XR rows; XI rows] x (R cols | S cols), x5x64, fp8
    W2 = (U2[indices] @ V2).reshape(nch, inf, OUT_F)
    sc = 5.0 * FP8_SCALE
    W2q = np.empty((nch, 2 * inf, 2 * OUT_F), np.float32)
    W2q[:, :inf, :OUT_F] = sc * W2.real      # R cols, XR rows
    W2q[:, inf:, :OUT_F] = -sc * W2.imag     # R cols, XI rows
    W2q[:, :inf, OUT_F:] = sc * W2.imag      # S cols, XR rows
    W2q[:, inf:, OUT_F:] = sc * W2.real      # S cols, XI rows
    w2 = W2q.reshape(nch, 2, 2, 128, 2 * OUT_F).transpose(0, 1, 3, 2, 4)
    w2 = np.ascontiguousarray(w2).astype(f8)  # [nch, c, p(128), i(2), j]

    # x: interleaved re/im rows bf16 (lin1) + blocked fp8 pair layout (lin2)
    xv = x.view(np.float32).reshape(nch, npts, 2 * inf)
    xt = np.ascontiguousarray(xv.transpose(0, 2, 1)).astype(bf16)  # [nch,512,npts]
    xT = x.transpose(0, 2, 1)
    XR = np.ascontiguousarray(xT.real.astype(np.float32))
    XI = np.ascontiguousarray(xT.imag.astype(np.float32))
    xq = np.stack([XR, XI], axis=1).reshape(nch, 2, 2, 128, npts)
    xq = np.ascontiguousarray(xq.transpose(0, 1, 3, 2, 4)).astype(f8)
    # [nch, c, p(128), i(2), npts]

    bg1 = b1[indices, 0, :]
    bg2 = b2[indices, 0, :]
    biasv = np.stack(
        [5.0 * bg1.real, 5.0 * bg1.imag + 3.0, 5.0 * bg2.real, 5.0 * bg2.imag,
         30.0 * bg1.real, np.zeros_like(bg1.real)],
        axis=1,
    ).astype(np.float32)                                    # [nch,6,256]
    return xt, xq, w1, w2, biasv


def make_in_maps(xt, xq, w1, w2, biasv):
    in_maps = []
    for c in range(NCORES):
        sl = slice(c * CH_PER_CORE, (c + 1) * CH_PER_CORE)
        in_maps.append(
            {
                "xt": np.ascontiguousarray(xt[sl]),
                "xq8": np.ascontiguousarray(xq[sl]),
                "w1": np.ascontiguousarray(w1[sl]),
                "w2": np.ascontiguousarray(w2[sl]),
                "biasv": np.ascontiguousarray(biasv[sl]),
            }
        )
    return in_maps


def combine_output(full, npts=NPTS):
    fr = np.asarray(full).astype(np.float32)
    out = (fr[:, 0] + 1j * fr[:, 1]).astype(np.complex64)
    return np.ascontiguousarray(out.transpose(0, 2, 1))


def kernel(x, indices, U1, V1, b1, U2, V2, b2):
    _ensure_path()
    from concourse.bass_utils import run_bass_kernel_spmd

    xt, xq, w1, w2, biasv = prepare_inputs(x, indices, U1, V1, b1, U2, V2, b2)
    nc = build_nc()
    in_maps = make_in_maps(xt, xq, w1, w2, biasv)
    res = run_bass_kernel_spmd(nc, in_maps, list(range(NCORES)))
    outs = [np.asarray(res.results[i]["out"]) for i in range(NCORES)]
    full = np.concatenate(outs, axis=0)
    return combine_output(full)


# revision 15
# speedup vs baseline: 1.3682x; 1.3682x over previous
"""Trainium2 Bass kernel for nn_AdaptiveMultiGabor2DLayer (v3).

Math (per channel c, ic = indices[c]):
    lin_l = x[c] @ W_l[ic] + b_l[ic]   (complex [NPTS, OUT_F]), l = 1,2
    out[c] = exp(i*30*lin1 - 25|lin1|^2 - 25|lin2|^2)
with p = 5Re lin1, q~ = 5Im lin1 + 3, r = 5Re lin2, s = 5Im lin2:
    out = C * erf'(p)erf'(q~)erf'(r)erf'(s) * (cos 6p + i sin 6p),
    C = e^9 (sqrt(pi)/2)^4,  erf'(z) = (2/sqrt(pi)) e^{-z^2}.

Structure per core (8 channels, expert parallel, no collectives):
  * lin1 (phase-critical) in bf16: P,Q directly in PSUM via K=512
    re/im-interleaved matmuls (4 chunks), exactly as the proven baseline.
  * lin2 (enters |.|^2 only - fp8 error contributes O(2 r dr) ~ 1e-3) via
    fp8e4 DoubleRow matmuls at half the PE cycles; weights pre-scaled by
    64*5 (fp8 subnormal avoidance), undone by the erf' scale=1/64.
  * Epilogue: 4x Derivative_Erf straight from PSUM (fused square+exp+bias),
    3 DVE 16-bit products for the gaussian envelope (e^9 folded via one
    scalar_tensor_tensor), full-angle trig: theta = 6P + 30Re(b1) staged
    in fp16, range-wrapped into [-pi,pi] by a 2x-enabled ADD_RANGE_WRAP
    clone (theta reaches +-4.4 rad, beyond the Sin table), one Sin each
    for sin/cos, and the two E*sin/E*cos products on the otherwise-idle
    GPSIMD engine.
  * ACT table sets: erf_derivative (G phase) / trig_and_small (T phase),
    batched over CH_BATCH channels -> only 2 table loads per batch.
"""

import math
import sys

import numpy as np

NCORES = 8
NCHAN = 64
NPTS = 2048
IN_F = 256
OUT_F = 256
CH_PER_CORE = NCHAN // NCORES
CH_BATCH = 4
PSW = 1024                 # psum quantity tile width (2 banks)
FP8_SCALE = 64.0
CBIG = math.exp(9.0) * (math.sqrt(math.pi) / 2.0) ** 4


def _ensure_path():
    try:
        import concourse  # noqa: F401
    except ImportError:
        for p in ("/opt/trn_rl_repo", "/root/.axon_site/_ro/trn_rl_repo"):
            if p not in sys.path:
                sys.path.insert(0, p)


_OPS_CACHE = {}


def _register_op(name, spec, perf_en=True):
    from concourse import dve_ops
    from concourse.dve_spec import _has_src1, lower
    from concourse.dve_uop import DveOpSpec

    existing = [o for o in dve_ops.OPS if o.name == name]
    if existing:
        return existing[0]
    row = max(dve_ops._SUB_OPCODE_FOR_NAME.values()) + 1
    assert row < 0x20
    dve_ops._SUB_OPCODE_FOR_NAME[name] = row
    shas = {}
    for ver in ("v3",):
        uops = lower(spec, ver=ver)
        shas[ver] = DveOpSpec(
            name=name, opcode=row, uops=uops, rd1_en=_has_src1(spec)
        ).sha(ver)
    op = dve_ops.DveOp(name, spec, subdim=False, uops_sha=shas,
                       perf_en={"v3": perf_en})
    dve_ops.OPS.append(op)
    dve_ops.CUSTOM_DVE_SPECS[name] = spec
    return op


def _get_arw():
    """ADD_RANGE_WRAP clone with the 16-bit perf slot enabled."""
    if "arw" in _OPS_CACHE:
        return _OPS_CACHE["arw"]
    _ensure_path()
    from concourse import dve_ops

    arw_op = _register_op("ARW2X_ANT", dve_ops.ADD_RANGE_WRAP.spec)
    _OPS_CACHE["arw"] = arw_op
    return arw_op


_NC_CACHE = {}


def build_nc(nch=CH_PER_CORE, npts=NPTS, ch_batch=CH_BATCH):
    key = (nch, npts, ch_batch)
    if key in _NC_CACHE:
        return _NC_CACHE[key]
    _ensure_path()
    import concourse.bacc as bacc
    import concourse.tile as tile
    from concourse import mybir

    arw_op = _get_arw()

    dt = mybir.dt
    AF = mybir.ActivationFunctionType
    OP = mybir.AluOpType
    PM = mybir.MatmulPerfMode

    n_pw = npts // PSW
    inv8 = 1.0 / FP8_SCALE

    nc = bacc.Bacc("TRN2", target_bir_lowering=False)
    xt_d = nc.declare_dram_parameter("xt", [nch, 512, npts], dt.bfloat16, isOutput=False)
    xq_d = nc.declare_dram_parameter("xq8", [nch, 2, 128, 2, npts], dt.float8e4, isOutput=False)
    w1_d = nc.declare_dram_parameter("w1", [nch, 4, 128, 512], dt.bfloat16, isOutput=False)
    w2_d = nc.declare_dram_parameter("w2", [nch, 2, 128, 2, 512], dt.float8e4, isOutput=False)
    bv_d = nc.declare_dram_parameter("biasv", [nch, 6, 256], dt.float32, isOutput=False)
    out_d = nc.declare_dram_parameter("out", [nch, 2, 256, npts], dt.bfloat16, isOutput=True)

    def bcol(ch, v, jh):
        return (ch * 6 + v) * 2 + jh

    n_stage = ch_batch * 2

    with tile.TileContext(nc) as tc:
        with (
            tc.tile_pool(name="xpool", bufs=2) as xpool,
            tc.tile_pool(name="wpool", bufs=2) as wpool,
            tc.tile_pool(name="cpool", bufs=1) as cpool,
            tc.tile_pool(name="spool", bufs=2) as spool,
            tc.tile_pool(name="stpool", bufs=1) as stpool,
            tc.tile_pool(name="pspool", bufs=2, space="PSUM") as pspool,
            tc.tile_pool(name="ps2pool", bufs=4, space="PSUM") as ps2pool,
        ):
            biast = cpool.tile([128, nch * 6 * 2], dt.float32)
            nc.sync.dma_start(
                out=biast[:], in_=bv_d[:].rearrange("c v (h p) -> p (c v h)", p=128)
            )

            def chain(inst):
                tc.chain_iter_dep("act_order", inst.ins if hasattr(inst, "ins") else inst)

            _cdma = {}

            def load_channel(ch):
                if ch in _cdma:
                    return _cdma[ch]
                xt_ks = [
                    xpool.tile([128, npts], dt.bfloat16, tag=f"xt{kc}", name=f"xt{kc}_{ch}")
                    for kc in range(4)
                ]
                xq = [xpool.tile([128, 2, npts], dt.float8e4, tag=f"xq{c}", name=f"xq{c}_{ch}")
                      for c in range(2)]
                w1 = wpool.tile([128, 4, 512], dt.bfloat16, tag="w1", name=f"w1_{ch}")
                w2 = wpool.tile([128, 2, 2, 512], dt.float8e4, tag="w2", name=f"w2_{ch}")
                nc.sync.dma_start(out=xt_ks[0][:], in_=xt_d[ch, 0:128, :])
                nc.sync.dma_start(
                    out=w1[:], in_=w1_d[ch].rearrange("c p j -> p c j")
                )
                for kc in range(1, 4):
                    nc.sync.dma_start(
                        out=xt_ks[kc][:], in_=xt_d[ch, kc * 128: kc * 128 + 128, :]
                    )
                for c in range(2):
                    nc.sync.dma_start(out=xq[c][:], in_=xq_d[ch, c])
                nc.sync.dma_start(
                    out=w2[:], in_=w2_d[ch].rearrange("c p i j -> p c i j")
                )
                _cdma[ch] = (xt_ks, xq, w1, w2)
                return _cdma[ch]

            def emit_unit_G(ch, jh, staged):
                xt_ks, xq, w1, w2 = load_channel(ch)
                th = stpool.tile([128, npts], dt.float16, tag="th", bufs=n_stage,
                                 name=f"th_{ch}_{jh}")
                mh = stpool.tile([128, npts], dt.bfloat16, tag="mh", bufs=n_stage,
                                 name=f"mh_{ch}_{jh}")
                for pw in range(n_pw):
                    psl = slice(pw * PSW, pw * PSW + PSW)
                    g = [spool.tile([128, PSW], dt.bfloat16, tag=f"g{qi}",
                                    name=f"g{qi}_{ch}_{jh}_{pw}") for qi in range(4)]
                    # P, Q: bf16 K=512 (4 chunks); j cols jh*128 (P) and
                    # 256 + jh*128 (Q) of the [512]-wide lin1 weight matrix
                    for qi, cb in ((0, jh * 128), (1, 256 + jh * 128)):
                        ps = pspool.tile([128, PSW], dt.float32, tag="q",
                                         name=f"ps{qi}_{ch}_{jh}_{pw}")
                        for ns in range(PSW // 512):
                            msl = slice(pw * PSW + ns * 512, pw * PSW + ns * 512 + 512)
                            for kc in range(4):
                                nc.tensor.matmul(
                                    ps[:, ns * 512: ns * 512 + 512],
                                    w1[:, kc, cb: cb + 128], xt_ks[kc][:, msl],
                                    start=(kc == 0), stop=(kc == 3))
                        inst = nc.scalar.activation(
                            g[qi][:], ps[:], AF.Derivative_Erf,
                            bias=biast[:, bcol(ch, qi, jh): bcol(ch, qi, jh) + 1])
                        chain(inst)
                        if qi == 0:
                            # theta = 6*P + 30Re(b1)  [fp16]
                            nc.vector.tensor_scalar(
                                out=th[:, psl], in0=ps[:], scalar1=6.0,
                                scalar2=biast[:, bcol(ch, 4, jh): bcol(ch, 4, jh) + 1],
                                op0=OP.mult, op1=OP.add)
                    # R, S: fp8 DoubleRow K=512 (2 chunks of 2x128),
                    # bank-aligned [128,512] psum tiles (DR at a psum column
                    # offset faults the exec unit)
                    for qi, cb in ((2, jh * 128), (3, 256 + jh * 128)):
                        for ns in range(PSW // 512):
                            msl = slice(pw * PSW + ns * 512, pw * PSW + ns * 512 + 512)
                            ps = ps2pool.tile([128, 512], dt.float32, tag="q2",
                                              name=f"ps{qi}_{ch}_{jh}_{pw}_{ns}")
                            for c in range(2):
                                nc.tensor.matmul(
                                    ps[:], w2[:, c, :, cb: cb + 128], xq[c][:, :, msl],
                                    start=(c == 0), stop=(c == 1),
                                    perf_mode=PM.DoubleRow)
                            inst = nc.scalar.activation(
                                g[qi][:, ns * 512: ns * 512 + 512], ps[:],
                                AF.Derivative_Erf, scale=inv8,
                                bias=biast[:, bcol(ch, qi, jh): bcol(ch, qi, jh) + 1])
                            chain(inst)
                    # envelope: mh = C * g0 g1 g2 g3  [16-bit 2x DVE]
                    m1 = spool.tile([128, PSW], dt.bfloat16, tag="m1", name=f"m1_{ch}_{jh}_{pw}")
                    m2 = spool.tile([128, PSW], dt.bfloat16, tag="m2", name=f"m2_{ch}_{jh}_{pw}")
                    nc.vector.tensor_tensor(out=m1[:], in0=g[0][:], in1=g[1][:], op=OP.mult)
                    nc.vector.tensor_tensor(out=m2[:], in0=g[2][:], in1=g[3][:], op=OP.mult)
                    nc.vector.scalar_tensor_tensor(
                        out=mh[:, psl], in0=m1[:], scalar=CBIG, in1=m2[:],
                        op0=OP.mult, op1=OP.mult)
                staged.append((ch, jh, th, mh))

            def emit_unit_T(ch, jh, th, mh):
                # sin directly on theta (|theta| <= 4.44; the Sin table is
                # good past 3.8 for all but ~1e-5 of elements, negligible in
                # the norm); cos needs theta+pi/2 (up to 6.0) -> one wrap.
                thc = spool.tile([128, npts], dt.float16, tag="thc", name=f"thc_{ch}_{jh}")
                nc.vector._custom_dve(arw_op, out=thc[:], in0=th[:], s0=math.pi / 2,
                                      s1=math.pi, imm2=2.0 * math.pi)
                sn = spool.tile([128, npts], dt.bfloat16, tag="sn", name=f"sn_{ch}_{jh}")
                cs = spool.tile([128, npts], dt.bfloat16, tag="cs", name=f"cs_{ch}_{jh}")
                zb = biast[:, bcol(ch, 5, jh): bcol(ch, 5, jh) + 1]
                inst = nc.scalar.activation(sn[:], th[:], AF.Sin, bias=zb)
                chain(inst)
                inst = nc.scalar.activation(cs[:], thc[:], AF.Sin, bias=zb)
                chain(inst)
                oii = spool.tile([128, npts], dt.bfloat16, tag="oii", name=f"oii_{ch}_{jh}")
                orr = spool.tile([128, npts], dt.bfloat16, tag="orr", name=f"orr_{ch}_{jh}")
                nc.gpsimd.tensor_tensor(out=oii[:], in0=mh[:], in1=sn[:], op=OP.mult)
                nc.gpsimd.tensor_tensor(out=orr[:], in0=mh[:], in1=cs[:], op=OP.mult)
                nc.sync.dma_start(out=out_d[ch, 0, jh * 128: jh * 128 + 128, :], in_=orr[:])
                nc.sync.dma_start(out=out_d[ch, 1, jh * 128: jh * 128 + 128, :], in_=oii[:])

            for b0 in range(0, nch, ch_batch):
                chs = list(range(b0, min(b0 + ch_batch, nch)))
                staged = []
                for ch in chs:
                    for jh in range(2):
                        emit_unit_G(ch, jh, staged)
                for (ch, jh, th, mh) in staged:
                    emit_unit_T(ch, jh, th, mh)

    nc.finalize()
    _NC_CACHE[key] = nc
    return nc


def prepare_inputs(x, indices, U1, V1, b1, U2, V2, b2):
    """Host marshaling. lin1: baseline interleaved-K bf16 form. lin2: fp8
    DoubleRow form scaled by 64*5."""
    import ml_dtypes

    bf16 = ml_dtypes.bfloat16
    f8 = ml_dtypes.float8_e4m3
    x = np.asarray(x)
    indices = np.asarray(indices).astype(np.int64)
    U1 = np.asarray(U1); V1 = np.asarray(V1); b1 = np.asarray(b1)
    U2 = np.asarray(U2); V2 = np.asarray(V2); b2 = np.asarray(b2)
    nch, npts, inf = x.shape

    # lin1 weights: [2*inf interleaved, 2*OUT_F] bf16, x5
    W1 = (U1[indices] @ V1).reshape(nch, inf, OUT_F)
    B = np.empty((nch, 2 * inf, 2 * OUT_F), np.float32)
    Wr = 5.0 * np.ascontiguousarray(W1.real)
    Wi = 5.0 * np.ascontiguousarray(W1.imag)
    B[:, 0::2, :OUT_F] = Wr
    B[:, 1::2, :OUT_F] = -Wi
    B[:, 0::2, OUT_F:] = Wi
    B[:, 1::2, OUT_F:] = Wr
    w1 = np.ascontiguousarray(B.reshape(nch, 4, 128, 2 * OUT_F)).astype(bf16)

    # lin2 weights: blocked [XR rows; XI rows] x (R cols | S cols), x5x64, fp8
    W2 = (U2[indices] @ V2).reshape(nch, inf, OUT_F)
    sc = 5.0 * FP8_SCALE
    W2q = np.empty((nch, 2 * inf, 2 * OUT_F), np.float32)
    W2q[:, :inf, :OUT_F] = sc * W2.real      # R cols, XR rows
    W2q[:, inf:, :OUT_F] = -sc * W2.imag     # R cols, XI rows
    W2q[:, :inf, OUT_F:] = sc * W2.imag      # S cols, XR rows
    W2q[:, inf:, OUT_F:] = sc * W2.real      # S cols, XI rows
    w2 = W2q.reshape(nch, 2, 2, 128, 2 * OUT_F).transpose(0, 1, 3, 2, 4)
    w2 = np.ascontiguousarray(w2).astype(f8)  # [nch, c, p(128), i(2), j]

    # x: interleaved re/im rows bf16 (lin1) + blocked fp8 pair layout (lin2)
    xv = x.view(np.float32).reshape(nch, npts, 2 * inf)
    xt = np.ascontiguousarray(xv.transpose(0, 2, 1)).astype(bf16)  # [nch,512,npts]
    xT = x.transpose(0, 2, 1)
    XR = np.ascontiguousarray(xT.real.astype(np.float32))
    XI = np.ascontiguousarray(xT.imag.astype(np.float32))
    xq = np.stack([XR, XI], axis=1).reshape(nch, 2, 2, 128, npts)
    xq = np.ascontiguousarray(xq.transpose(0, 1, 3, 2, 4)).astype(f8)
    # [nch, c, p(128), i(2), npts]

    bg1 = b1[indices, 0, :]
    bg2 = b2[indices, 0, :]
    biasv = np.stack(
        [5.0 * bg1.real, 5.0 * bg1.imag + 3.0, 5.0 * bg2.real, 5.0 * bg2.imag,
         30.0 * bg1.real, np.zeros_like(bg1.real)],
        axis=1,
    ).astype(np.float32)                                    # [nch,6,256]
    return xt, xq, w1, w2, biasv


def make_in_maps(xt, xq, w1, w2, biasv):
    in_maps = []
    for c in range(NCORES):
        sl = slice(c * CH_PER_CORE, (c + 1) * CH_PER_CORE)
        in_maps.append(
            {
                "xt": np.ascontiguousarray(xt[sl]),
                "xq8": np.ascontiguousarray(xq[sl]),
                "w1": np.ascontiguousarray(w1[sl]),
                "w2": np.ascontiguousarray(w2[sl]),
                "biasv": np.ascontiguousarray(biasv[sl]),
            }
        )
    return in_maps


def combine_output(full, npts=NPTS):
    fr = np.asarray(full).astype(np.float32)
    out = (fr[:, 0] + 1j * fr[:, 1]).astype(np.complex64)
    return np.ascontiguousarray(out.transpose(0, 2, 1))


def kernel(x, indices, U1, V1, b1, U2, V2, b2):
    _ensure_path()
    from concourse.bass_utils import run_bass_kernel_spmd

    xt, xq, w1, w2, biasv = prepare_inputs(x, indices, U1, V1, b1, U2, V2, b2)
    nc = build_nc()
    in_maps = make_in_maps(xt, xq, w1, w2, biasv)
    res = run_bass_kernel_spmd(nc, in_maps, list(range(NCORES)))
    outs = [np.asarray(res.results[i]["out"]) for i in range(NCORES)]
    full = np.concatenate(outs, axis=0)
    return combine_output(full)
